# revision 18
# baseline (speedup 1.0000x reference)
"""Exact sliding-window causal attention for Trainium2, sharded over 8 NeuronCores.

Problem: B=16, S=4096, D=64, window=512, causal. Each core handles 2 batches
(batch-parallel sharding, no cross-core communication).

Per-core algorithm:
  - q,k are cast to bf16 (DRAM->DRAM SWDGE cast into a 128-col padded scratch)
    and transposed into [d, s] layout with one xbar DMA-transpose each
    (the contraction dim must sit on partitions for the QK^T matmul).
  - v is cast to bf16 into SBUF with a constant ones column appended at index D,
    so the PV matmul also produces the softmax denominator.
  - For each 128-wide key tile kt, the queries that attend to it are the five
    tiles qt in [kt, kt+4]. One [128,512] + one [128,128] matmul produce
    scores^T (keys on partitions, queries on free dim) in fp32 PSUM.
  - exp on the scalar engine (scores ~ N(0,1) so no max-subtraction is needed),
    writing bf16 probs; the two triangular masks (causal diagonal tile and
    sliding-window far tile) are applied post-exp by gpsimd.affine_select
    (multiplicative 0/1, exact).
  - Each query tile's 5 PV matmuls run as one consecutive fp32-PSUM
    accumulation group (probs stationary, [v|1] moving), then a [128,1]
    reciprocal of the denominator + tensor_scalar_mul normalizes into the
    output tile. All reductions/normalization stay fp32.
"""

import sys

if "/opt/trn_rl_repo" not in sys.path:
    sys.path.insert(0, "/opt/trn_rl_repo")

from contextlib import ExitStack

import numpy as np

import concourse.bass as bass
import concourse.tile as tile
from concourse import mybir
from concourse.bass_utils import run_bass_kernel_spmd

B, S, D = 16, 4096, 64
WINDOW = 512
N_CORES = 8
B_LOCAL = B // N_CORES  # batches per core
NT = S // 128  # 32 query/key tiles per batch
F32 = mybir.dt.float32
BF16 = mybir.dt.bfloat16


# --- workaround: this walrus build accepts at most ONE sync-wait command per
# instruction. After Tile has assigned semaphores, hoist excess waits onto
# same-engine NoOp instructions inserted immediately before the instruction
# (the engine blocks on each in program order — identical semantics).
def _split_multi_waits(nc, max_waits=1):
    n_split = 0
    for f in nc.m.functions:
        for bb in f.blocks:
            insts = bb.instructions
            i = 0
            while i < len(insts):
                inst = insts[i]
                si = inst.sync_info
                if si is not None and si.on_wait and len(si.on_wait) > max_waits:
                    waits = list(si.on_wait)
                    si.on_wait = waits[-max_waits:]
                    for jj, w in enumerate(waits[:-max_waits]):
                        nop = mybir.InstNoOp(
                            name=f"{inst.name}_wnop{jj}", ins=[], outs=[]
                        )
                        nop.engine = inst.engine
                        nop.sync_info = mybir.SyncInfo(on_wait=[w], on_update=[])
                        insts.insert(i, nop)
                        i += 1
                        n_split += 1
                i += 1
    return n_split


def build():
    nc = bass.Bass(
        "TRN2", target_bir_lowering=False, debug=False, num_devices=N_CORES
    )
    q = nc.dram_tensor("q", [B_LOCAL, S, D], F32, kind="ExternalInput").ap()
    k = nc.dram_tensor("k", [B_LOCAL, S, D], F32, kind="ExternalInput").ap()
    v = nc.dram_tensor("v", [B_LOCAL, S, D], F32, kind="ExternalInput").ap()
    out = nc.dram_tensor("out", [B_LOCAL, S, D], F32, kind="ExternalOutput").ap()
    scale = float(D) ** -0.5

    with tile.TileContext(nc) as tc, ExitStack() as ctx:
        singles = ctx.enter_context(tc.tile_pool(name="singles", bufs=1))
        tqp = ctx.enter_context(tc.tile_pool(name="tqp", bufs=2))
        probsp = ctx.enter_context(tc.tile_pool(name="probsp", bufs=7))
        outp = ctx.enter_context(tc.tile_pool(name="outp", bufs=2))
        recp = ctx.enter_context(tc.tile_pool(name="recp", bufs=4))
        dramp = ctx.enter_context(tc.tile_pool(name="dramp", bufs=2, space="DRAM"))
        psp = ctx.enter_context(tc.tile_pool(name="psp", bufs=2, space="PSUM"))
        accp = ctx.enter_context(tc.tile_pool(name="accp", bufs=3, space="PSUM"))

        # v with a ones column at index D: the PV matmul then yields the
        # softmax denominator in accumulator column D. memset once; DMAs only
        # overwrite the first D columns each batch.
        v_ext = []
        for i in range(min(2, B_LOCAL)):
            ve = singles.tile([128, NT, D + 2], BF16, tag=f"vext{i}")
            nc.vector.memset(ve, 1.0)
            v_ext.append(ve)

        for b in range(B_LOCAL):
            ve = v_ext[b % 2]
            # bf16 cast in DRAM (padded to 128 cols so the xbar transpose's
            # free-dim multiple-of-128 constraint holds), then one DMA
            # transpose per tensor into [d, s] layout. Partitions 64..127 of
            # qT/kT hold the padding garbage and are never read.
            qpad = dramp.tile([S, 128], BF16, tag="qpad")
            kpad = dramp.tile([S, 128], BF16, tag="kpad")
            nc.gpsimd.dma_start(qpad[:, 0:D], q[b])
            nc.gpsimd.dma_start(kpad[:, 0:D], k[b])
            qT = tqp.tile([128, S], BF16, tag="qT")
            kT = tqp.tile([128, S], BF16, tag="kT")
            nc.sync.dma_start_transpose(qT[:], qpad[:])
            nc.sync.dma_start_transpose(kT[:], kpad[:])
            nc.gpsimd.dma_start(
                ve[:, :, 0:D], v[b].rearrange("(n p) d -> p n d", p=128)
            )

            out_sb = outp.tile([128, NT * D], F32)
            probs_hist = {}

            def emit_pv(qt, b=b, ve=ve, out_sb=out_sb, probs_hist=probs_hist):
                lo = max(0, qt - 4)
                acc = accp.tile(
                    [128, D + 2], F32, tag="acc", name=f"acc_{b}_{qt}"
                )
                for kt2 in range(lo, qt + 1):
                    pt = probs_hist[kt2]
                    s = qt - kt2  # slot of qt within probs[kt2]
                    nc.tensor.matmul(
                        acc[:],
                        lhsT=pt[:, s * 128 : (s + 1) * 128],
                        rhs=ve[:, kt2, :],
                        start=(kt2 == lo),
                        stop=(kt2 == qt),
                    )
                probs_hist.pop(qt - 4, None)
                rec = recp.tile([128, 1], F32, tag="rec", name=f"rec_{b}_{qt}")
                nc.vector.reciprocal(rec, acc[:, D : D + 1])
                nc.vector.tensor_scalar_mul(
                    out_sb[:, qt * D : (qt + 1) * D],
                    acc[:, 0:D],
                    rec,
                )

            for kt in range(NT):
                hi = min(kt + 4, NT - 1)
                nslots = hi - kt + 1  # slots s=0.. map to qt=kt+s
                w4 = min(4, nslots) * 128
                scores = psp.tile([128, 640], F32, tag="ps")
                kT_t = kT[0:64, kt * 128 : (kt + 1) * 128]
                nc.tensor.matmul(
                    scores[:, 0:w4],
                    lhsT=kT_t,
                    rhs=qT[0:64, kt * 128 : kt * 128 + w4],
                    start=True,
                    stop=True,
                )
                if nslots == 5:
                    nc.tensor.matmul(
                        scores[:, 512:640],
                        lhsT=kT_t,
                        rhs=qT[0:64, (kt + 4) * 128 : (kt + 5) * 128],
                        start=True,
                        stop=True,
                    )
                probs = probsp.tile([128, 640], BF16)
                nc.scalar.activation(
                    probs[:, 0 : nslots * 128],
                    scores[:, 0 : nslots * 128],
                    mybir.ActivationFunctionType.Exp,
                    scale=scale,
                )
                # diagonal tile (qt == kt): keep keys j <= queries i
                # layout is scores^T: partition=j within tile, free=i
                nc.gpsimd.affine_select(
                    out=probs[:, 0:128],
                    in_=probs[:, 0:128],
                    compare_op=mybir.AluOpType.is_ge,
                    fill=0.0,
                    base=0,
                    pattern=[[1, 128]],
                    channel_multiplier=-1,
                )
                if nslots == 5:
                    # far tile (qt == kt+4): window start, keep j >= i+1
                    nc.gpsimd.affine_select(
                        out=probs[:, 512:640],
                        in_=probs[:, 512:640],
                        compare_op=mybir.AluOpType.is_ge,
                        fill=0.0,
                        base=-1,
                        pattern=[[-1, 128]],
                        channel_multiplier=1,
                    )
                probs_hist[kt] = probs
                # software pipeline: run PV for query tile qt=kt-1 (whose probs
                # all completed last iteration) so the in-order PE queue can
                # start this iteration's QK without waiting on ACT/gpsimd.
                if kt >= 1:
                    emit_pv(kt - 1)
            emit_pv(NT - 1)
            nc.sync.dma_start(
                out[b].rearrange("(n p) d -> p n d", p=128),
                out_sb.rearrange("p (n d) -> p n d", d=D),
            )
    _split_multi_waits(nc)
    return nc


_CACHE = {}


def _get_nc():
    if "nc" not in _CACHE:
        _CACHE["nc"] = build()
    return _CACHE["nc"]


def _make_in_maps(q, k, v):
    q = np.ascontiguousarray(np.asarray(q, dtype=np.float32))
    k = np.ascontiguousarray(np.asarray(k, dtype=np.float32))
    v = np.ascontiguousarray(np.asarray(v, dtype=np.float32))
    return [
        {
            "q": np.ascontiguousarray(q[c * B_LOCAL : (c + 1) * B_LOCAL]),
            "k": np.ascontiguousarray(k[c * B_LOCAL : (c + 1) * B_LOCAL]),
            "v": np.ascontiguousarray(v[c * B_LOCAL : (c + 1) * B_LOCAL]),
        }
        for c in range(N_CORES)
    ]


def kernel(q, k, v):
    nc = _get_nc()
    res = run_bass_kernel_spmd(nc, _make_in_maps(q, k, v), core_ids=list(range(N_CORES)))
    return np.concatenate(
        [res.results[c]["out"] for c in range(N_CORES)], axis=0
    )


# revision 19
# speedup vs baseline: 1.2367x; 1.2367x over previous
"""Exact sliding-window causal attention for Trainium2, sharded over 8 NeuronCores.

Problem: B=16, S=4096, D=64, window=512, causal. Each core handles 2 batches
(batch-parallel sharding, no cross-core communication).

Per-core algorithm:
  - q,k are cast to bf16 (DRAM->DRAM SWDGE cast into a 128-col padded scratch)
    and transposed into [d, s] layout with one xbar DMA-transpose each
    (the contraction dim must sit on partitions for the QK^T matmul).
  - v is cast to bf16 into SBUF with a constant ones column appended at index D,
    so the PV matmul also produces the softmax denominator.
  - For each 128-wide key tile kt, the queries that attend to it are the five
    tiles qt in [kt, kt+4]. One [128,512] + one [128,128] matmul produce
    scores^T (keys on partitions, queries on free dim) in fp32 PSUM.
  - exp on the scalar engine (scores ~ N(0,1) so no max-subtraction is needed),
    writing bf16 probs; the two triangular masks (causal diagonal tile and
    sliding-window far tile) are applied post-exp by gpsimd.affine_select
    (multiplicative 0/1, exact).
  - Each query tile's 5 PV matmuls run as one consecutive fp32-PSUM
    accumulation group (probs stationary, [v|1] moving), then a [128,1]
    reciprocal of the denominator + tensor_scalar_mul normalizes into the
    output tile. All reductions/normalization stay fp32.
"""

import sys

if "/opt/trn_rl_repo" not in sys.path:
    sys.path.insert(0, "/opt/trn_rl_repo")

from contextlib import ExitStack

import numpy as np

import concourse.bass as bass
import concourse.tile as tile
from concourse import mybir
from concourse.bass_utils import run_bass_kernel_spmd

B, S, D = 16, 4096, 64
WINDOW = 512
N_CORES = 8
B_LOCAL = B // N_CORES  # batches per core
NT = S // 128  # 32 query/key tiles per batch
F32 = mybir.dt.float32
BF16 = mybir.dt.bfloat16


# --- workaround: this walrus build accepts at most ONE sync-wait command per
# instruction. After Tile has assigned semaphores, hoist excess waits onto
# same-engine NoOp instructions inserted immediately before the instruction
# (the engine blocks on each in program order — identical semantics).
def _split_multi_waits(nc, max_waits=1):
    n_split = 0
    for f in nc.m.functions:
        for bb in f.blocks:
            insts = bb.instructions
            i = 0
            while i < len(insts):
                inst = insts[i]
                si = inst.sync_info
                if si is not None and si.on_wait and len(si.on_wait) > max_waits:
                    waits = list(si.on_wait)
                    si.on_wait = waits[-max_waits:]
                    for jj, w in enumerate(waits[:-max_waits]):
                        nop = mybir.InstNoOp(
                            name=f"{inst.name}_wnop{jj}", ins=[], outs=[]
                        )
                        nop.engine = inst.engine
                        nop.sync_info = mybir.SyncInfo(on_wait=[w], on_update=[])
                        insts.insert(i, nop)
                        i += 1
                        n_split += 1
                i += 1
    return n_split


def build():
    nc = bass.Bass(
        "TRN2", target_bir_lowering=False, debug=False, num_devices=N_CORES
    )
    q = nc.dram_tensor("q", [B_LOCAL, S, D], F32, kind="ExternalInput").ap()
    k = nc.dram_tensor("k", [B_LOCAL, S, D], F32, kind="ExternalInput").ap()
    v = nc.dram_tensor("v", [B_LOCAL, S, D], F32, kind="ExternalInput").ap()
    out = nc.dram_tensor("out", [B_LOCAL, S, D], F32, kind="ExternalOutput").ap()
    scale = float(D) ** -0.5

    with tile.TileContext(nc) as tc, ExitStack() as ctx:
        singles = ctx.enter_context(tc.tile_pool(name="singles", bufs=1))
        tqp = ctx.enter_context(tc.tile_pool(name="tqp", bufs=2))
        probsp = ctx.enter_context(tc.tile_pool(name="probsp", bufs=7))
        outp = ctx.enter_context(tc.tile_pool(name="outp", bufs=2))
        vldp = ctx.enter_context(tc.tile_pool(name="vldp", bufs=2))
        recp = ctx.enter_context(tc.tile_pool(name="recp", bufs=4))
        dramp = ctx.enter_context(tc.tile_pool(name="dramp", bufs=2, space="DRAM"))
        psp = ctx.enter_context(tc.tile_pool(name="psp", bufs=3, space="PSUM"))
        accp = ctx.enter_context(tc.tile_pool(name="accp", bufs=2, space="PSUM"))

        # v with a ones column at index D: the PV matmul then yields the
        # softmax denominator in accumulator column D. memset once; DMAs only
        # overwrite the first D columns each batch.
        v_ext = []
        for i in range(min(2, B_LOCAL)):
            ve = singles.tile([128, NT, D + 2], BF16, tag=f"vext{i}")
            nc.vector.memset(ve, 1.0)
            v_ext.append(ve)

        # stage all batches' inputs up front so batch b+1's DMAs overlap
        # batch b's compute. Batch 0 is chunked so its first QK can start
        # after ~1/4 of the transpose instead of the whole thing.
        qTs, kTs, ves = [], [], []
        for b in range(B_LOCAL):
            ve = v_ext[b % 2]
            # bf16 cast in DRAM (padded to 128 cols so the xbar transpose's
            # free-dim multiple-of-128 constraint holds), then one DMA
            # transpose per tensor into [d, s] layout. Partitions 64..127 of
            # qT/kT hold the padding garbage and are never read.
            qpad = dramp.tile([S, 128], BF16, tag="qpad")
            kpad = dramp.tile([S, 128], BF16, tag="kpad")
            qT = tqp.tile([128, S], BF16, tag="qT")
            kT = tqp.tile([128, S], BF16, tag="kT")
            nch = 4 if b == 0 else 1
            c = S // nch
            for i in range(nch):
                nc.gpsimd.dma_start(qpad[i * c : (i + 1) * c, 0:D], q[b][i * c : (i + 1) * c])
                nc.sync.dma_start_transpose(
                    qT[:, i * c : (i + 1) * c], qpad[i * c : (i + 1) * c, :]
                )
                nc.gpsimd.dma_start(kpad[i * c : (i + 1) * c, 0:D], k[b][i * c : (i + 1) * c])
                nc.sync.dma_start_transpose(
                    kT[:, i * c : (i + 1) * c], kpad[i * c : (i + 1) * c, :]
                )
            # v: fast fp32 HWDGE load + DVE cast into the ones-extended tile
            v_sb = vldp.tile([128, NT, D], F32, tag="vsb")
            nc.sync.dma_start(v_sb[:], v[b].rearrange("(n p) d -> p n d", p=128))
            nc.vector.tensor_copy(ve[:, :, 0:D], v_sb[:])
            qTs.append(qT)
            kTs.append(kT)
            ves.append(ve)

        for b in range(B_LOCAL):
            ve = ves[b]
            qT = qTs[b]
            kT = kTs[b]
            out_sb = outp.tile([128, NT * D], F32)
            probs_hist = {}

            def emit_pv(qt, b=b, ve=ve, out_sb=out_sb, probs_hist=probs_hist):
                lo = max(0, qt - 4)
                acc = accp.tile(
                    [128, D + 2], F32, tag="acc", name=f"acc_{b}_{qt}"
                )
                for kt2 in range(lo, qt + 1):
                    pt = probs_hist[kt2]
                    s = qt - kt2  # slot of qt within probs[kt2]
                    nc.tensor.matmul(
                        acc[:],
                        lhsT=pt[:, s * 128 : (s + 1) * 128],
                        rhs=ve[:, kt2, :],
                        start=(kt2 == lo),
                        stop=(kt2 == qt),
                    )
                probs_hist.pop(qt - 4, None)
                rec = recp.tile([128, 1], F32, tag="rec", name=f"rec_{b}_{qt}")
                nc.vector.reciprocal(rec, acc[:, D : D + 1])
                nc.vector.tensor_scalar_mul(
                    out_sb[:, qt * D : (qt + 1) * D],
                    acc[:, 0:D],
                    rec,
                )

            for kt in range(NT):
                hi = min(kt + 4, NT - 1)
                nslots = hi - kt + 1  # slots s=0.. map to qt=kt+s
                w4 = min(4, nslots) * 128
                scores = psp.tile([128, 640], F32, tag="ps")
                kT_t = kT[0:64, kt * 128 : (kt + 1) * 128]
                nc.tensor.matmul(
                    scores[:, 0:w4],
                    lhsT=kT_t,
                    rhs=qT[0:64, kt * 128 : kt * 128 + w4],
                    start=True,
                    stop=True,
                )
                if nslots == 5:
                    nc.tensor.matmul(
                        scores[:, 512:640],
                        lhsT=kT_t,
                        rhs=qT[0:64, (kt + 4) * 128 : (kt + 5) * 128],
                        start=True,
                        stop=True,
                    )
                probs = probsp.tile([128, 640], BF16)
                nc.scalar.activation(
                    probs[:, 0 : nslots * 128],
                    scores[:, 0 : nslots * 128],
                    mybir.ActivationFunctionType.Exp,
                    scale=scale,
                )
                # diagonal tile (qt == kt): keep keys j <= queries i
                # layout is scores^T: partition=j within tile, free=i
                nc.gpsimd.affine_select(
                    out=probs[:, 0:128],
                    in_=probs[:, 0:128],
                    compare_op=mybir.AluOpType.is_ge,
                    fill=0.0,
                    base=0,
                    pattern=[[1, 128]],
                    channel_multiplier=-1,
                )
                if nslots == 5:
                    # far tile (qt == kt+4): window start, keep j >= i+1
                    nc.gpsimd.affine_select(
                        out=probs[:, 512:640],
                        in_=probs[:, 512:640],
                        compare_op=mybir.AluOpType.is_ge,
                        fill=0.0,
                        base=-1,
                        pattern=[[-1, 128]],
                        channel_multiplier=1,
                    )
                probs_hist[kt] = probs
                # software pipeline: run PV for query tile qt=kt-1 (whose probs
                # all completed last iteration) so the in-order PE queue can
                # start this iteration's QK without waiting on ACT/gpsimd.
                if kt >= 1:
                    emit_pv(kt - 1)
            emit_pv(NT - 1)
            nc.sync.dma_start(
                out[b].rearrange("(n p) d -> p n d", p=128),
                out_sb.rearrange("p (n d) -> p n d", d=D),
            )
    _split_multi_waits(nc)
    return nc


_CACHE = {}


def _get_nc():
    if "nc" not in _CACHE:
        _CACHE["nc"] = build()
    return _CACHE["nc"]


def _make_in_maps(q, k, v):
    q = np.ascontiguousarray(np.asarray(q, dtype=np.float32))
    k = np.ascontiguousarray(np.asarray(k, dtype=np.float32))
    v = np.ascontiguousarray(np.asarray(v, dtype=np.float32))
    return [
        {
            "q": np.ascontiguousarray(q[c * B_LOCAL : (c + 1) * B_LOCAL]),
            "k": np.ascontiguousarray(k[c * B_LOCAL : (c + 1) * B_LOCAL]),
            "v": np.ascontiguousarray(v[c * B_LOCAL : (c + 1) * B_LOCAL]),
        }
        for c in range(N_CORES)
    ]


def kernel(q, k, v):
    nc = _get_nc()
    res = run_bass_kernel_spmd(nc, _make_in_maps(q, k, v), core_ids=list(range(N_CORES)))
    return np.concatenate(
        [res.results[c]["out"] for c in range(N_CORES)], axis=0
    )
